# revision 1
# baseline (speedup 1.0000x reference)
"""DeepCell GNN message-passing kernel for 8 Trainium2 NeuronCores.

Levelized DAG recurrence. Design:
  - State table Tn [N+8, 256] bf16 in node order holds [hs | hf] per node,
    replicated per core (Internal+Shared DRAM); level-0 rows host-initialized.
  - Per level l: nodes split into 8 contiguous chunks (one per core). Each core:
      * dma_gather (transposed, bf16, local int16 indices into the window
        [lo_l, start_l) of Tn) pulls hs^T/hf^T for its cross-level edges.
      * structural MLP (3 layers) and functional MLP on transposed activations;
        layer 3 is emitted per-128-edge chunk with the activations as the
        stationary operand so the product lands row-major (edge-major) in PSUM.
      * segment-sum via one-hot matmul into per-window PSUM accumulators
        [128 h, <=512 nodes]; same-level edges (sources still zero) and the
        layer-3 bias are folded in as a K=2 rank-1 matmul with host-computed
        [mlp(0); b3] against [same_count; cross_indeg].
      * GRU with h_prev = 0 (nodes are written exactly once, at their level).
      * updated [hs|hf] rows AllGather'ed into Tn[start_l : start_l + 8*J_l).
  - hf output is written per core as transposed f32 columns; host reassembles.
"""

import math

import numpy as np
import ml_dtypes

import concourse.bass as bass
import concourse.bacc as bacc
import concourse.mybir as mybir
import concourse.tile as tile
from concourse.bass_utils import run_bass_kernel_spmd

NCORES = 8
P = 128
WIN = 512          # scatter/GRU window width (nodes)
GCHUNK = 512       # edges per dma_gather
MGROUP = 512       # edges per MLP group
f32 = mybir.dt.float32
bf16 = mybir.dt.bfloat16
fp16 = mybir.dt.float16
i16 = mybir.dt.int16

BF = ml_dtypes.bfloat16


def _ceil(a, b):
    return -(-a // b)


def _rup(a, b):
    return _ceil(a, b) * b


def _mlp_np(h, w1, b1, w2, b2, w3, b3):
    h = np.maximum(h @ w1 + b1, 0.0)
    h = np.maximum(h @ w2 + b2, 0.0)
    return h @ w3 + b3


def _prep(inputs):
    x = np.asarray(inputs["x"], np.float32)
    ei = np.asarray(inputs["edge_index"], np.int64)
    fl = np.asarray(inputs["forward_level"], np.int64)
    n = x.shape[0]
    dh = 128
    num_levels = int(fl.max()) + 1

    start = np.searchsorted(fl, np.arange(num_levels + 1)).astype(np.int64)
    src_all, tgt_all = ei[0], ei[1]
    tlv = fl[tgt_all]
    slv = fl[src_all]
    keep = tlv >= 1
    same = keep & (slv == tlv)
    cross = keep & (slv < tlv)
    cnt_same = np.bincount(tgt_all[same], minlength=n).astype(np.int64)
    cnt_cross = np.bincount(tgt_all[cross], minlength=n).astype(np.int64)

    # cross edges sorted by (dst) -> grouped per level automatically (fl sorted)
    cs, ct = src_all[cross], tgt_all[cross]
    order = np.argsort(ct, kind="stable")
    cs, ct = cs[order], ct[order]
    ct_lv = fl[ct]
    # per-level slices of the cross edge stream
    lvl_edge_start = np.searchsorted(ct_lv, np.arange(num_levels + 1))

    levels = []
    idx_cols = 0
    dst_cols = 0
    j_cols = 0
    for l in range(1, num_levels):
        s_l, e_l = int(start[l]), int(start[l + 1])
        n_l = e_l - s_l
        if n_l == 0:
            continue
        J = _ceil(n_l, NCORES)
        JP = _rup(J, P)
        widths = []
        rem = JP
        while rem > 0:
            w = min(WIN, rem)
            widths.append(w)
            rem -= w
        nw = len(widths)
        es, ee = int(lvl_edge_start[l]), int(lvl_edge_start[l + 1])
        esrc, edst = cs[es:ee], ct[es:ee]
        lo = int(esrc.min()) if ee > es else max(0, s_l - 1)
        span = s_l - lo
        assert 0 < span < 32767, f"level {l}: src span {span} exceeds int16"
        # per (core, window) edge lists
        k_of = (edst - s_l) // J
        loc = (edst - s_l) - k_of * J
        w_of = loc // WIN
        per_kw = [[None] * nw for _ in range(NCORES)]
        for k in range(NCORES):
            mk = k_of == k
            for w in range(nw):
                m = mk & (w_of == w)
                per_kw[k][w] = (esrc[m], loc[m] - w * WIN)
        Pw = [
            _rup(max(per_kw[k][w][0].size for k in range(NCORES)), P) // P
            for w in range(nw)
        ]  # tiles per window
        T = sum(Pw)
        T4 = _rup(max(T, 0), GCHUNK // 128)
        Pw[nw - 1] += T4 - T  # absorb gather-chunk alignment into last window
        T = T4
        # tile -> window map
        tile_win = []
        for w in range(nw):
            tile_win += [w] * Pw[w]
        levels.append(
            dict(
                l=l, s=s_l, n=n_l, J=J, JP=JP, widths=widths, nw=nw, Pw=Pw, T=T,
                lo=lo, span=span, per_kw=per_kw,
                idx_off=idx_cols, dst_off=dst_cols, j_off=j_cols,
                tile_win=tile_win,
            )
        )
        idx_cols += (T // 4) * (GCHUNK // 16)
        dst_cols += T
        j_cols += JP

    # ---- per-core packed arrays ----
    percore = []
    for k in range(NCORES):
        idx16 = np.zeros((P, max(idx_cols, 1)), np.int16)
        dstc = np.full((P, max(dst_cols, 1)), -1.0, np.float32)
        cnt2 = np.zeros((2, max(j_cols, 1)), np.float16)
        xT = np.zeros((64 if x.shape[1] <= 64 else x.shape[1], max(j_cols, 1)), BF)
        for lv in levels:
            T, nw, Pw = lv["T"], lv["nw"], lv["Pw"]
            lo = lv["lo"]
            # edge stream for this core at this level
            e_idx = np.zeros(T * P, np.int16)
            e_dst = np.full(T * P, -1.0, np.float32)
            pos = 0
            for w in range(nw):
                es_, dl = lv["per_kw"][k][w]
                cnt_e = es_.size
                e_idx[pos : pos + cnt_e] = (es_ - lo).astype(np.int16)
                e_dst[pos : pos + cnt_e] = dl.astype(np.float32)
                pos += Pw[w] * P
            # pack dstcol [128, T]
            dstc[:, lv["dst_off"] : lv["dst_off"] + T] = e_dst.reshape(T, P).T
            # pack idx16: per gather chunk of 512, layout [i%16, i//16], tiled x8
            nch = T * P // GCHUNK
            blk = e_idx.reshape(nch, GCHUNK // 16, 16)  # [chunk, col, lane]
            blk = np.transpose(blk, (2, 0, 1)).reshape(16, nch * (GCHUNK // 16))
            idx16[:, lv["idx_off"] : lv["idx_off"] + nch * (GCHUNK // 16)] = np.tile(
                blk, (8, 1)
            )
            # cnt2 + xT
            a_k = lv["s"] + k * lv["J"]
            real = max(0, min(lv["J"], lv["n"] - k * lv["J"]))
            jo = lv["j_off"]
            if real > 0:
                rows = np.arange(a_k, a_k + real)
                cnt2[0, jo : jo + real] = cnt_same[rows].astype(np.float16)
                cnt2[1, jo : jo + real] = cnt_cross[rows].astype(np.float16)
                xT[: x.shape[1], jo : jo + real] = x[rows].T.astype(BF)
        percore.append(dict(idx16=idx16, dstcol=dstc, cnt2=cnt2, xT=xT))

    # ---- level-0 init ----
    import jax

    n0 = int(start[1])
    cpu = jax.devices("cpu")[0]
    with jax.default_device(cpu):
        v = jax.random.uniform(jax.random.key(1), (n0, dh), np.float32) - 0.5
        v = v / np.linalg.norm(np.asarray(v), axis=1, keepdims=True)
    hs0 = np.asarray(v, np.float32)
    init0 = np.zeros((max(n0, 1), 256), BF)
    init0[:n0, :dh] = hs0.astype(BF)

    # ---- weights ----
    g = lambda name: np.asarray(inputs[name], np.float32)
    sw1, sw2, sw3 = g("sw1"), g("sw2"), g("sw3")
    sb1, sb2, sb3 = g("sb1"), g("sb2"), g("sb3")
    fw1, fw2, fw3 = g("fw1"), g("fw2"), g("fw3")
    fb1, fb2, fb3 = g("fb1"), g("fb2"), g("fb3")
    c_s = _mlp_np(np.zeros((1, dh), np.float32), sw1, sb1, sw2, sb2, sw3, sb3)[0]
    c_f = _mlp_np(np.zeros((1, 2 * dh), np.float32), fw1, fb1, fw2, fb2, fw3, fb3)[0]
    cb3 = np.zeros((2, 256), np.float16)
    cb3[0, :dh] = c_s.astype(np.float16)
    cb3[1, :dh] = sb3.astype(np.float16)
    cb3[0, dh:] = c_f.astype(np.float16)
    cb3[1, dh:] = fb3.astype(np.float16)

    def gru_pack(wih, whh, bih, bhh):
        wih = np.asarray(wih, np.float32)
        bih = np.asarray(bih, np.float32)
        bhh = np.asarray(bhh, np.float32)
        wT_h = wih[:, :dh].T.astype(BF)          # [128, 384]
        wT_x = wih[:, dh:].T.astype(BF)          # [64, 384]
        bias = np.zeros((P, 4), np.float32)
        bias[:, 0] = bih[0:dh] + bhh[0:dh]       # r
        bias[:, 1] = -(bih[dh : 2 * dh] + bhh[dh : 2 * dh])  # -z bias (sigma(-x) trick)
        bias[:, 2] = bih[2 * dh : 3 * dh]        # i_n bias
        bias[:, 3] = bhh[2 * dh : 3 * dh]        # h_n bias (scaled by r)
        return wT_h, wT_x, bias

    gs_wTh, gs_wTx, gs_bias = gru_pack(
        inputs["gs_wih"], inputs["gs_whh"], inputs["gs_bih"], inputs["gs_bhh"]
    )
    gf_wTh, gf_wTx, gf_bias = gru_pack(
        inputs["gf_wih"], inputs["gf_whh"], inputs["gf_bih"], inputs["gf_bhh"]
    )

    mlp_bias = np.zeros((P, 4), np.float32)
    mlp_bias[:, 0], mlp_bias[:, 1] = sb1, sb2
    mlp_bias[:, 2], mlp_bias[:, 3] = fb1, fb2

    weights = dict(
        init0=init0,
        sw1=sw1.astype(BF), sw2=sw2.astype(BF), sw3=sw3.astype(BF),
        fw1=fw1.astype(BF), fw2=fw2.astype(BF), fw3=fw3.astype(BF),
        cb3=cb3, mlp_bias=mlp_bias,
        gs_wTh=gs_wTh, gs_wTx=gs_wTx, gs_bias=gs_bias,
        gf_wTh=gf_wTh, gf_wTx=gf_wTx, gf_bias=gf_bias,
    )

    meta = dict(
        n=n, dh=dh, dx=x.shape[1], n0=n0, levels=levels,
        idx_cols=max(idx_cols, 1), dst_cols=max(dst_cols, 1),
        j_cols=max(j_cols, 1),
    )
    return meta, percore, weights


def _build(meta, emit_ag=True, emit_gather=True, tail='pe', meta_eng='scalar'):
    n, dh, dx = meta["n"], meta["dh"], meta["dx"]
    n0 = meta["n0"]
    nc = bacc.Bacc("TRN2", target_bir_lowering=False, debug=False, num_devices=NCORES)

    init0_d = nc.dram_tensor("init0", [max(n0, 1), 256], bf16, kind="ExternalInput")
    idx_d = nc.dram_tensor("idx16", [P, meta["idx_cols"]], i16, kind="ExternalInput")
    dst_d = nc.dram_tensor("dstcol", [P, meta["dst_cols"]], f32, kind="ExternalInput")
    cnt_d = nc.dram_tensor("cnt2", [2, meta["j_cols"]], fp16, kind="ExternalInput")
    xT_d = nc.dram_tensor("xT", [64, meta["j_cols"]], bf16, kind="ExternalInput")
    w_d = {}
    for nm, shp, dt in [
        ("sw1", [dh, dh], bf16), ("sw2", [dh, dh], bf16), ("sw3", [dh, dh], bf16),
        ("fw1", [2 * dh, dh], bf16), ("fw2", [dh, dh], bf16), ("fw3", [dh, dh], bf16),
        ("cb3", [2, 256], fp16), ("mlp_bias", [P, 4], f32),
        ("gs_wTh", [dh, 3 * dh], bf16), ("gs_wTx", [dx, 3 * dh], bf16),
        ("gs_bias", [P, 4], f32),
        ("gf_wTh", [dh, 3 * dh], bf16), ("gf_wTx", [dx, 3 * dh], bf16),
        ("gf_bias", [P, 4], f32),
    ]:
        w_d[nm] = nc.dram_tensor(nm, shp, dt, kind="ExternalInput")
    out_d = nc.dram_tensor("out_hfT", [P, meta["j_cols"]], f32, kind="ExternalOutput")
    Tn = nc.dram_tensor("Tn", [n + 8, 256], bf16, addr_space="Shared")

    Jmax = max((lv["JP"] for lv in meta["levels"]), default=P)

    with tile.TileContext(nc) as tc:
        with (
            tc.tile_pool(name="consts", bufs=1) as cst,
            tc.tile_pool(name="sb", bufs=2) as sb,
            tc.tile_pool(name="psA", bufs=2, space="PSUM") as psA,
            tc.tile_pool(name="psB", bufs=2, space="PSUM") as psB,
            tc.tile_pool(name="psC", bufs=2, space="PSUM") as psC,
            tc.tile_pool(name="dram", bufs=2, space="DRAM") as drp,
        ):
            # --- consts ---
            wt = {}
            for nm in ("sw1", "sw2", "sw3", "fw2", "fw3"):
                wt[nm] = cst.tile([dh, dh], bf16, tag=nm, name=nm)
                nc.sync.dma_start(out=wt[nm][:], in_=w_d[nm][:, :])
            wt["fw1a"] = cst.tile([dh, dh], bf16, tag="fw1a", name="fw1a")
            wt["fw1b"] = cst.tile([dh, dh], bf16, tag="fw1b", name="fw1b")
            nc.sync.dma_start(out=wt["fw1a"][:], in_=w_d["fw1"][0:dh, :])
            nc.sync.dma_start(out=wt["fw1b"][:], in_=w_d["fw1"][dh : 2 * dh, :])
            for nm in ("gs_wTh", "gf_wTh"):
                wt[nm] = cst.tile([dh, 3 * dh], bf16, tag=nm, name=nm)
                nc.sync.dma_start(out=wt[nm][:], in_=w_d[nm][:, :])
            for nm in ("gs_wTx", "gf_wTx"):
                wt[nm] = cst.tile([dx, 3 * dh], bf16, tag=nm, name=nm)
                nc.sync.dma_start(out=wt[nm][:], in_=w_d[nm][:, :])
            for nm in ("mlp_bias", "gs_bias", "gf_bias"):
                wt[nm] = cst.tile([P, 4], f32, tag=nm, name=nm)
                nc.sync.dma_start(out=wt[nm][:], in_=w_d[nm][:, :])
            wt["cb3"] = cst.tile([2, 256], fp16, tag="cb3", name="cb3")
            nc.sync.dma_start(out=wt["cb3"][:], in_=w_d["cb3"][:, :])

            from concourse.masks import make_identity
            ident = cst.tile([P, P], f32, tag="ident", name="ident")
            make_identity(nc, ident[:])
            iota_i = cst.tile([P, WIN], mybir.dt.int32, tag="iota_i", name="iota_i")
            nc.gpsimd.iota(iota_i[:], pattern=[[1, WIN]], base=0, channel_multiplier=0)
            iota_f = cst.tile([P, WIN], f32, tag="iota_f", name="iota_f")
            nc.vector.tensor_copy(iota_f[:], iota_i[:])

            mb = wt["mlp_bias"]
            # --- init level 0 rows ---
            nc.sync.dma_start(out=Tn[0:n0, :], in_=init0_d[0:n0, :])

            for lv in meta["levels"]:
                s_l, J, JP = lv["s"], lv["J"], lv["JP"]
                lo, T, nw = lv["lo"], lv["T"], lv["nw"]
                widths, Pw = lv["widths"], lv["Pw"]
                tile_win = lv["tile_win"]
                jo = lv["j_off"]

                # level metadata loads
                dst_sb = sb.tile([P, max(T, 1)], f32, tag="dst_sb", name="dst_sb")
                meng = nc.scalar if meta_eng == 'scalar' else nc.sync
                if T > 0:
                    meng.dma_start(
                        out=dst_sb[:], in_=dst_d[:, lv["dst_off"] : lv["dst_off"] + T]
                    )
                nidx = (T // 4) * (GCHUNK // 16)
                idx_sb = sb.tile([P, max(nidx, 1)], i16, tag="idx_sb", name="idx_sb")
                if nidx > 0:
                    meng.dma_start(
                        out=idx_sb[:], in_=idx_d[:, lv["idx_off"] : lv["idx_off"] + nidx]
                    )
                cnt_sb = sb.tile([2, JP], fp16, tag="cnt_sb", name="cnt_sb")
                meng.dma_start(out=cnt_sb[:], in_=cnt_d[:, jo : jo + JP])
                xT_sb = sb.tile([64, JP], bf16, tag="xT_sb", name="xT_sb")
                meng.dma_start(out=xT_sb[:], in_=xT_d[:, jo : jo + JP])

                ag_in = drp.tile([Jmax, 256], bf16, tag="ag_in", name="ag_in")
                rm_sb = sb.tile([P, Jmax // P, 256], bf16, tag="rm_sb", name="rm_sb")
                hfout = sb.tile([P, Jmax], f32, tag="hfout", name="hfout")

                # window psum accumulators (rank-1 emitted lazily at first use)
                wps = [None] * nw
                wlast = []  # last tile index per window
                tpos = 0
                for w in range(nw):
                    tpos += Pw[w]
                    wlast.append(tpos - 1)

                def start_window(w):
                    pS = psA.tile([P, WIN], f32, tag="msgaccS", name="msgaccS")
                    pF = psA.tile([P, WIN], f32, tag="msgaccF", name="msgaccF")
                    wd = widths[w]
                    only = Pw[w] == 0
                    nc.tensor.matmul(
                        out=pS[:, :wd], lhsT=wt["cb3"][:, 0:dh],
                        rhs=cnt_sb[:, w * WIN : w * WIN + wd],
                        start=True, stop=only,
                    )
                    nc.tensor.matmul(
                        out=pF[:, :wd], lhsT=wt["cb3"][:, dh : 2 * dh],
                        rhs=cnt_sb[:, w * WIN : w * WIN + wd],
                        start=True, stop=only,
                    )
                    wps[w] = (pS, pF)

                def gru(w):
                    wd = widths[w]
                    woff = w * WIN
                    pS, pF = wps[w]
                    for st, pm in (("s", pS), ("f", pF)):
                        wTh = wt["gs_wTh" if st == "s" else "gf_wTh"]
                        wTx = wt["gs_wTx" if st == "s" else "gf_wTx"]
                        gb = wt["gs_bias" if st == "s" else "gf_bias"]
                        msgT = sb.tile([P, WIN], bf16, tag="msgT", name="msgT")
                        nc.scalar.activation(
                            msgT[:, :wd], pm[:, :wd], mybir.ActivationFunctionType.Copy
                        )
                        pg = []
                        for gi in range(3):
                            pgi = psC.tile([P, WIN], f32, tag="gates", name="gates")
                            gsl = slice(gi * dh, (gi + 1) * dh)
                            nc.tensor.matmul(
                                out=pgi[:, :wd], lhsT=wTh[:, gsl], rhs=msgT[:, :wd],
                                start=True, stop=False,
                            )
                            nc.tensor.matmul(
                                out=pgi[:, :wd], lhsT=wTx[:dx, gsl],
                                rhs=xT_sb[:dx, woff : woff + wd],
                                start=False, stop=True,
                            )
                            pg.append(pgi)
                        r_sb = sb.tile([P, WIN], f32, tag="r_sb", name="r_sb")
                        nc.scalar.activation(
                            r_sb[:, :wd], pg[0][:, :wd],
                            mybir.ActivationFunctionType.Sigmoid, bias=gb[:, 0:1],
                        )
                        z_sb = sb.tile([P, WIN], f32, tag="z_sb", name="z_sb")
                        nc.scalar.activation(
                            z_sb[:, :wd], pg[1][:, :wd],
                            mybir.ActivationFunctionType.Sigmoid, bias=gb[:, 1:2],
                            scale=-1.0,
                        )
                        rb = sb.tile([P, WIN], f32, tag="rb", name="rb")
                        nc.vector.tensor_scalar_mul(rb[:, :wd], r_sb[:, :wd], gb[:, 3:4])
                        npre = sb.tile([P, WIN], f32, tag="npre", name="npre")
                        nc.vector.tensor_tensor(
                            out=npre[:, :wd], in0=rb[:, :wd], in1=pg[2][:, :wd],
                            op=mybir.AluOpType.add,
                        )
                        n_sb = sb.tile([P, WIN], f32, tag="n_sb", name="n_sb")
                        nc.scalar.activation(
                            n_sb[:, :wd], npre[:, :wd],
                            mybir.ActivationFunctionType.Tanh, bias=gb[:, 2:3],
                        )
                        # h = n * sigma(-(i_z+b_z))  [z_sb already holds 1-z]
                        if st == "f":
                            hN_ap = hfout[:, woff : woff + wd]
                        else:
                            hNs = sb.tile([P, WIN], f32, tag="hNs", name="hNs")
                            hN_ap = hNs[:, :wd]
                        nc.vector.tensor_tensor(
                            out=hN_ap, in0=n_sb[:, :wd], in1=z_sb[:, :wd],
                            op=mybir.AluOpType.mult,
                        )
                        nb = wd // P
                        csl = slice(0, dh) if st == "s" else slice(dh, 2 * dh)
                        if tail == 'pe':
                            tp = psB.tile([P, WIN], f32, tag="mlp", name="tp")
                            for b in range(nb):
                                nc.tensor.transpose(
                                    out=tp[:, b * P : (b + 1) * P],
                                    in_=hN_ap[:, b * P : (b + 1) * P]
                                    if st == "s"
                                    else hfout[:, woff + b * P : woff + (b + 1) * P],
                                    identity=ident[:],
                                )
                            dst_rm = rm_sb[:, w * (WIN // P) : w * (WIN // P) + nb, csl]
                            if (w + (0 if st == "s" else 1)) % 2 == 0:
                                nc.scalar.activation(
                                    dst_rm, tp[:, : nb * P],
                                    mybir.ActivationFunctionType.Copy,
                                )
                            else:
                                nc.vector.tensor_copy(dst_rm, tp[:, : nb * P])
                        else:
                            hNb = sb.tile([P, WIN], bf16, tag="hNb", name="hNb")
                            nc.scalar.activation(
                                hNb[:, :wd],
                                hN_ap if st == "s" else hfout[:, woff : woff + wd],
                                mybir.ActivationFunctionType.Copy,
                            )
                            rm = sb.tile([P, WIN // P, P], bf16, tag="rmx", name="rmx")
                            for b in range(nb):
                                nc.sync.dma_start_transpose(
                                    out=rm[:, b, :], in_=hNb[:, b * P : (b + 1) * P]
                                )
                            nc.sync.dma_start(
                                out=ag_in[woff : woff + wd, csl].rearrange(
                                    "(b p) h -> p b h", p=P
                                ),
                                in_=rm[:, :nb, :],
                            )

                # --- edge groups ---
                for g in range(T // 4):
                    gc, gsub = divmod(g, GCHUNK // MGROUP)
                    if gsub == 0:
                        gth = sb.tile([P, 2, GCHUNK], bf16, tag="gth", name="gth")
                        if emit_gather:
                            nc.gpsimd.dma_gather(
                                out_ap=gth[:],
                                in_ap=Tn[lo:s_l, :],
                                idxs_ap=idx_sb[
                                    :, gc * (GCHUNK // 16) : (gc + 1) * (GCHUNK // 16)
                                ],
                                num_idxs=GCHUNK,
                                num_idxs_reg=GCHUNK,
                                elem_size=256,
                                transpose=True,
                            )
                        else:
                            base = min(lo, s_l - GCHUNK) if s_l - GCHUNK >= 0 else 0
                            nc.sync.dma_start(
                                out=gth[:],
                                in_=Tn[base : base + GCHUNK, :].rearrange(
                                    "(a p) c -> p a c", p=P
                                ),
                            )
                    hsT = gth[:, 0, gsub * MGROUP : (gsub + 1) * MGROUP]
                    hfT = gth[:, 1, gsub * MGROUP : (gsub + 1) * MGROUP]
                    # structural MLP
                    p1 = psB.tile([P, MGROUP], f32, tag="mlp", name="mlp")
                    nc.tensor.matmul(out=p1[:], lhsT=wt["sw1"][:], rhs=hsT)
                    h1 = sb.tile([P, MGROUP], bf16, tag="h1", name="h1")
                    nc.scalar.activation(
                        h1[:], p1[:], mybir.ActivationFunctionType.Relu, bias=mb[:, 0:1]
                    )
                    p2 = psB.tile([P, MGROUP], f32, tag="mlp", name="mlp")
                    nc.tensor.matmul(out=p2[:], lhsT=wt["sw2"][:], rhs=h1[:])
                    h2 = sb.tile([P, MGROUP], bf16, tag="h2", name="h2")
                    nc.scalar.activation(
                        h2[:], p2[:], mybir.ActivationFunctionType.Relu, bias=mb[:, 1:2]
                    )
                    p3 = psB.tile([P, MGROUP], f32, tag="mlp", name="mlp")
                    for t4 in range(4):
                        sl = slice(t4 * P, (t4 + 1) * P)
                        nc.tensor.matmul(
                            out=p3[:, sl], lhsT=h2[:, sl], rhs=wt["sw3"][:]
                        )
                    msgS = sb.tile([P, MGROUP], bf16, tag="msgS", name="msgS")
                    nc.vector.tensor_copy(msgS[:], p3[:])
                    # functional MLP
                    q1 = psB.tile([P, MGROUP], f32, tag="mlp", name="mlp")
                    nc.tensor.matmul(
                        out=q1[:], lhsT=wt["fw1a"][:], rhs=hsT, start=True, stop=False
                    )
                    nc.tensor.matmul(
                        out=q1[:], lhsT=wt["fw1b"][:], rhs=hfT, start=False, stop=True
                    )
                    f1 = sb.tile([P, MGROUP], bf16, tag="f1", name="f1")
                    nc.scalar.activation(
                        f1[:], q1[:], mybir.ActivationFunctionType.Relu, bias=mb[:, 2:3]
                    )
                    q2 = psB.tile([P, MGROUP], f32, tag="mlp", name="mlp")
                    nc.tensor.matmul(out=q2[:], lhsT=wt["fw2"][:], rhs=f1[:])
                    f2 = sb.tile([P, MGROUP], bf16, tag="f2", name="f2")
                    nc.scalar.activation(
                        f2[:], q2[:], mybir.ActivationFunctionType.Relu, bias=mb[:, 3:4]
                    )
                    q3 = psB.tile([P, MGROUP], f32, tag="mlp", name="mlp")
                    for t4 in range(4):
                        sl = slice(t4 * P, (t4 + 1) * P)
                        nc.tensor.matmul(
                            out=q3[:, sl], lhsT=f2[:, sl], rhs=wt["fw3"][:]
                        )
                    msgF = sb.tile([P, MGROUP], bf16, tag="msgF", name="msgF")
                    nc.vector.tensor_copy(msgF[:], q3[:])
                    # scatter per 128-edge tile
                    for t4 in range(4):
                        t = g * 4 + t4
                        w = tile_win[t]
                        if wps[w] is None:
                            start_window(w)
                        wd = widths[w]
                        S = sb.tile([P, WIN], bf16, tag="onehot", name="onehot")
                        nc.vector.tensor_scalar(
                            S[:, :wd], iota_f[:, :wd], dst_sb[:, t : t + 1], None,
                            op0=mybir.AluOpType.is_equal,
                        )
                        last = t == wlast[w]
                        sl = slice(t4 * P, (t4 + 1) * P)
                        pS, pF = wps[w]
                        nc.tensor.matmul(
                            out=pS[:, :wd], lhsT=msgS[:, sl], rhs=S[:, :wd],
                            start=False, stop=last,
                        )
                        nc.tensor.matmul(
                            out=pF[:, :wd], lhsT=msgF[:, sl], rhs=S[:, :wd],
                            start=False, stop=last,
                        )
                        if last:
                            gru(w)
                # windows with zero tiles
                for w in range(nw):
                    if wps[w] is None:
                        start_window(w)
                        gru(w)

                if tail == 'pe':
                    nbl = JP // P
                    nc.sync.dma_start(
                        out=ag_in[0:JP, :].rearrange("(b p) c -> p b c", p=P),
                        in_=rm_sb[:, :nbl, :],
                    )
                nc.sync.dma_start(out=out_d[:, jo : jo + JP], in_=hfout[:, :JP])
                if emit_ag:
                    nc.gpsimd.collective_compute(
                        "AllGather",
                        mybir.AluOpType.bypass,
                        replica_groups=[list(range(NCORES))],
                        ins=[ag_in[0:J, :]],
                        outs=[Tn[s_l : s_l + NCORES * J, :]],
                    )
                else:
                    nc.sync.dma_start(
                        out=Tn[s_l : s_l + J, :], in_=ag_in[0:J, :]
                    )
    nc.compile()
    return nc


def _assemble(meta, results):
    n, dh = meta["n"], meta["dh"]
    hf = np.zeros((n, dh), np.float32)
    for lv in meta["levels"]:
        for k in range(NCORES):
            real = max(0, min(lv["J"], lv["n"] - k * lv["J"]))
            if real == 0:
                continue
            a_k = lv["s"] + k * lv["J"]
            cols = results[k]["out_hfT"][:, lv["j_off"] : lv["j_off"] + real]
            hf[a_k : a_k + real] = cols.T
    return hf


def build_and_run(inputs, trace=False, **kwargs):
    meta, percore, weights = _prep(inputs)
    nc = _build(meta)
    in_maps = [dict(percore[c], **weights) for c in range(NCORES)]
    res = run_bass_kernel_spmd(
        nc, in_maps, core_ids=list(range(NCORES)), trace=trace, **kwargs
    )
    return _assemble(meta, res.results), res


def kernel(**inputs):
    out, _ = build_and_run(inputs)
    return out



# revision 11
# speedup vs baseline: 4.5582x; 4.5582x over previous
"""DeepCell GNN message-passing kernel for 8 Trainium2 NeuronCores.

Levelized DAG recurrence (all cross edges source the immediately previous
level). v4 design — per-level targeted AllToAll instead of AllGather:

  - Nodes of each level are strided across the 8 cores (position p -> core
    p%8, local index p//8), flattening the per-(sender,reader) edge counts so
    every level's A2A stripe is E=128 edge slots.
  - Each core keeps its own level-l states [hs|hf] bf16 node-major in SBUF
    (rm_sb).  At level end it dma_gathers (SBUF-source) the per-edge source
    states every reader needs, in the reader's slot order, dumps them to an
    internal DRAM buffer and runs a collective AllToAll (8 stripes x 128
    slots x 512B = 0.5MB) instead of AllGather-ing all states (3.2MB).
  - Receivers consume their A2A stripes directly: the received buffer IS the
    per-edge [hs^T|hf^T] input of the message MLPs — no receiver gather.
  - Structural/functional MLPs, one-hot segment-sum scatter into per-window
    PSUM accumulators, GRU with h_prev=0, and the constant fold of
    same-level edges ([mlp(0); b3] x [cnt_same; cnt_cross]) are unchanged
    from the baseline kernel.
  - Level-1's "received" stripes are host-computed from the level-0 init
    states and fed as an input; levels 1..22 emit an A2A for the next level.
  - hf output is written per core as transposed f32 columns; host reassembles.
"""

import numpy as np
import ml_dtypes

import concourse.bass as bass
import concourse.bacc as bacc
import concourse.mybir as mybir
import concourse.tile as tile
from concourse.bass_utils import run_bass_kernel_spmd

NCORES = 8
P = 128
WIN = 512          # scatter/GRU window width (nodes)
MGROUP = 512       # edges per MLP group
f32 = mybir.dt.float32
bf16 = mybir.dt.bfloat16
fp16 = mybir.dt.float16
i16 = mybir.dt.int16

BF = ml_dtypes.bfloat16


def _ceil(a, b):
    return -(-a // b)


def _rup(a, b):
    return _ceil(a, b) * b


def _mlp_np(h, w1, b1, w2, b2, w3, b3):
    h = np.maximum(h @ w1 + b1, 0.0)
    h = np.maximum(h @ w2 + b2, 0.0)
    return h @ w3 + b3


def _pack_idx16(idx, ncols):
    """Pack an int16 index vector (len multiple of 16) into the
    [16-row wrapped, replicated to 128 partitions] dma_gather format."""
    n = idx.size
    assert n % 16 == 0 and n // 16 == ncols
    blk = idx.reshape(ncols, 16).T  # [16, ncols]; idx i -> row i%16, col i//16
    return np.tile(blk, (8, 1))     # [128, ncols]


def _prep(inputs):
    x = np.asarray(inputs["x"], np.float32)
    ei = np.asarray(inputs["edge_index"], np.int64)
    fl = np.asarray(inputs["forward_level"], np.int64)
    n = x.shape[0]
    dh = 128
    num_levels = int(fl.max()) + 1

    start = np.searchsorted(fl, np.arange(num_levels + 1)).astype(np.int64)
    src_all, tgt_all = ei[0], ei[1]
    tlv = fl[tgt_all]
    slv = fl[src_all]
    keep = tlv >= 1
    same = keep & (slv == tlv)
    cross = keep & (slv < tlv)
    assert (slv[cross] == tlv[cross] - 1).all(), "cross edges must source prev level"
    cnt_same = np.bincount(tgt_all[same], minlength=n).astype(np.int64)
    cnt_cross = np.bincount(tgt_all[cross], minlength=n).astype(np.int64)

    cs, ct = src_all[cross], tgt_all[cross]
    ct_lv = fl[ct]

    # ---- per-level structure (strided node->core assignment) ----
    levels = []
    idx_cols = 0
    dst_cols = 0
    j_cols = 0
    for l in range(1, num_levels):
        s_l, e_l = int(start[l]), int(start[l + 1])
        n_l = e_l - s_l
        assert n_l > 0
        J = _ceil(n_l, NCORES)
        JP = _rup(J, P)
        widths = []
        rem = JP
        while rem > 0:
            w = min(WIN, rem)
            widths.append(w)
            rem -= w
        m = ct_lv == l
        es, et = cs[m], ct[m]
        p_t = et - s_l                  # position within level l
        p_s = es - int(start[l - 1])    # position within level l-1
        rd = (p_t % NCORES).astype(np.int64)   # reader core
        sd = (p_s % NCORES).astype(np.int64)   # sender core
        d_loc = (p_t // NCORES).astype(np.int64)
        s_loc = (p_s // NCORES).astype(np.int64)
        pair = np.bincount(sd * NCORES + rd, minlength=NCORES * NCORES)
        E = _rup(max(int(pair.max()), 1), P)
        # per (sender k, reader j): slot lists
        send_idx = [np.zeros(NCORES * E, np.int16) for _ in range(NCORES)]
        dst_slot = [np.full(NCORES * E, -1.0, np.float32) for _ in range(NCORES)]
        order = np.lexsort((d_loc, rd, sd))
        es_o = order  # edges sorted by (sender, reader, dst)
        sd_o, rd_o = sd[es_o], rd[es_o]
        sl_o, dl_o = s_loc[es_o], d_loc[es_o]
        # slot within (k, j) group = running count
        key = sd_o * NCORES + rd_o
        # running position within each key group
        grp_start = np.searchsorted(key, np.arange(NCORES * NCORES))
        pos_in_grp = np.arange(key.size) - grp_start[key]
        slot = rd_o * E + pos_in_grp  # slot in SENDER k's send layout
        for k in range(NCORES):
            mk = sd_o == k
            send_idx[k][slot[mk]] = sl_o[mk].astype(np.int16)
        rslot = sd_o * E + pos_in_grp  # slot in READER j's receive layout
        for j in range(NCORES):
            mj = rd_o == j
            dst_slot[j][rslot[mj]] = dl_o[mj].astype(np.float32)
        T = NCORES * E // P
        # (tile, window) scatter descriptors: union across reader cores so the
        # emitted program (and dst_sb column order) is identical on all cores
        union = []
        for t in range(T):
            for w, wd in enumerate(widths):
                lo, hi = w * WIN, w * WIN + wd
                hit = False
                for j in range(NCORES):
                    dvals = dst_slot[j][t * P : (t + 1) * P]
                    if ((dvals >= lo) & (dvals < hi)).any():
                        hit = True
                        break
                if hit:
                    union.append((t, w))
        ntw = max(len(union), 1)
        levels.append(
            dict(
                l=l, s=s_l, n=n_l, J=J, JP=JP, widths=widths, nw=len(widths),
                E=E, T=T, send_idx=send_idx, dst_slot=dst_slot, union=union,
                ntw=ntw,
                idx_off=idx_cols, dst_off=dst_cols, j_off=j_cols,
            )
        )
        if l >= 2:
            # sender gather for this level's stripes is emitted at level l-1
            idx_cols += NCORES * E // 16
        dst_cols += ntw
        j_cols += JP

    # ---- per-core packed arrays ----
    percore = []
    for k in range(NCORES):
        idx16 = np.zeros((P, max(idx_cols, 1)), np.int16)
        dstc = np.full((P, max(dst_cols, 1)), -1.0, np.float32)
        cnt2 = np.zeros((2, max(j_cols, 1)), np.float16)
        xT = np.zeros((64, max(j_cols, 1)), BF)
        for lv in levels:
            l, E, T = lv["l"], lv["E"], lv["T"]
            if l >= 2:
                idx16[:, lv["idx_off"] : lv["idx_off"] + NCORES * E // 16] = (
                    _pack_idx16(lv["send_idx"][k], NCORES * E // 16)
                )
            # dst columns in the shared union order
            cols = np.full((P, lv["ntw"]), -1.0, np.float32)
            for ci, (t, w) in enumerate(lv["union"]):
                dv = lv["dst_slot"][k][t * P : (t + 1) * P].copy()
                lo, hi = w * WIN, w * WIN + lv["widths"][w]
                out = (dv < lo) | (dv >= hi)
                dv = dv - lo
                dv[out] = -1.0
                cols[:, ci] = dv
            dstc[:, lv["dst_off"] : lv["dst_off"] + lv["ntw"]] = cols
            # cnt2 + xT (strided rows)
            real = lv["n"] // NCORES + (1 if k < lv["n"] % NCORES else 0)
            jo = lv["j_off"]
            if real > 0:
                rows = lv["s"] + np.arange(real) * NCORES + k
                cnt2[0, jo : jo + real] = cnt_same[rows].astype(np.float16)
                cnt2[1, jo : jo + real] = cnt_cross[rows].astype(np.float16)
                xT[:, jo : jo + real] = x[rows].T.astype(BF)
        percore.append(dict(idx16=idx16, dstcol=dstc, cnt2=cnt2, xT=xT))

    # ---- level-0 init & level-1 receive stripes ----
    import jax

    n0 = int(start[1])
    cpu = jax.devices("cpu")[0]
    with jax.default_device(cpu):
        v = jax.random.uniform(jax.random.key(1), (n0, dh), np.float32) - 0.5
        v = v / np.linalg.norm(np.asarray(v), axis=1, keepdims=True)
    hs0 = np.asarray(v, np.float32)

    lv1 = levels[0]
    E1 = lv1["E"]
    for j in range(NCORES):
        buf = np.zeros((P, 2, NCORES, E1), BF)
        # reader j's stripe from sender k: source states of its (k, j) edges
        m = (ct_lv == 1)
        es1, et1 = cs[m], ct[m]
        p_t = et1 - lv1["s"]
        p_s = es1  # level-0 positions ARE global indices
        rdj = (p_t % NCORES) == j
        sdj = (p_s % NCORES).astype(np.int64)[rdj]
        s_glob = es1[rdj]
        d_loc = (p_t // NCORES)[rdj]
        o = np.lexsort((d_loc, sdj))
        sdj, s_glob = sdj[o], s_glob[o]
        gs = np.searchsorted(sdj, np.arange(NCORES))
        pos = np.arange(sdj.size) - gs[sdj]
        sl = sdj * E1 + pos
        st = hs0[s_glob].astype(BF)  # [cnt, 128]
        buf[:, 0, sl // E1, sl % E1] = st.T
        percore[j]["recv0"] = buf.reshape(P, 2 * NCORES * E1)

    # ---- weights ----
    g = lambda name: np.asarray(inputs[name], np.float32)
    sw1, sw2, sw3 = g("sw1"), g("sw2"), g("sw3")
    sb1, sb2, sb3 = g("sb1"), g("sb2"), g("sb3")
    fw1, fw2, fw3 = g("fw1"), g("fw2"), g("fw3")
    fb1, fb2, fb3 = g("fb1"), g("fb2"), g("fb3")
    c_s = _mlp_np(np.zeros((1, dh), np.float32), sw1, sb1, sw2, sb2, sw3, sb3)[0]
    c_f = _mlp_np(np.zeros((1, 2 * dh), np.float32), fw1, fb1, fw2, fb2, fw3, fb3)[0]
    cb3 = np.zeros((2, 256), np.float16)
    cb3[0, :dh] = c_s.astype(np.float16)
    cb3[1, :dh] = sb3.astype(np.float16)
    cb3[0, dh:] = c_f.astype(np.float16)
    cb3[1, dh:] = fb3.astype(np.float16)

    def gru_pack(wih, whh, bih, bhh):
        wih = np.asarray(wih, np.float32)
        bih = np.asarray(bih, np.float32)
        bhh = np.asarray(bhh, np.float32)
        wT_h = wih[:, :dh].T.astype(BF)          # [128, 384]
        wT_x = wih[:, dh:].T.astype(BF)          # [64, 384]
        bias = np.zeros((P, 4), np.float32)
        bias[:, 0] = bih[0:dh] + bhh[0:dh]       # r
        bias[:, 1] = -(bih[dh : 2 * dh] + bhh[dh : 2 * dh])  # -z bias
        bias[:, 2] = bih[2 * dh : 3 * dh]        # i_n bias
        bias[:, 3] = bhh[2 * dh : 3 * dh]        # h_n bias (scaled by r)
        return wT_h, wT_x, bias

    gs_wTh, gs_wTx, gs_bias = gru_pack(
        inputs["gs_wih"], inputs["gs_whh"], inputs["gs_bih"], inputs["gs_bhh"]
    )
    gf_wTh, gf_wTx, gf_bias = gru_pack(
        inputs["gf_wih"], inputs["gf_whh"], inputs["gf_bih"], inputs["gf_bhh"]
    )

    mlp_bias = np.zeros((P, 4), np.float32)
    mlp_bias[:, 0], mlp_bias[:, 1] = sb1, sb2
    mlp_bias[:, 2], mlp_bias[:, 3] = fb1, fb2

    weights = dict(
        sw1=sw1.astype(BF), sw2=sw2.astype(BF), sw3=sw3.astype(BF),
        fw1=fw1.astype(BF), fw2=fw2.astype(BF), fw3=fw3.astype(BF),
        cb3=cb3, mlp_bias=mlp_bias,
        gs_wTh=gs_wTh, gs_wTx=gs_wTx, gs_bias=gs_bias,
        gf_wTh=gf_wTh, gf_wTx=gf_wTx, gf_bias=gf_bias,
    )

    meta = dict(
        n=n, dh=dh, dx=64, n0=n0, levels=levels,
        idx_cols=max(idx_cols, 1), dst_cols=max(dst_cols, 1),
        j_cols=max(j_cols, 1), E1=E1,
    )
    return meta, percore, weights


def _build(meta):
    dh, dx = meta["dh"], meta["dx"]
    levels = meta["levels"]
    nc = bacc.Bacc("TRN2", target_bir_lowering=False, debug=False, num_devices=NCORES)

    idx_d = nc.dram_tensor("idx16", [P, meta["idx_cols"]], i16, kind="ExternalInput")
    dst_d = nc.dram_tensor("dstcol", [P, meta["dst_cols"]], f32, kind="ExternalInput")
    cnt_d = nc.dram_tensor("cnt2", [2, meta["j_cols"]], fp16, kind="ExternalInput")
    xT_d = nc.dram_tensor("xT", [64, meta["j_cols"]], bf16, kind="ExternalInput")
    E1 = meta["E1"]
    recv0_d = nc.dram_tensor(
        "recv0", [P, 2 * NCORES * E1], bf16, kind="ExternalInput"
    )
    w_d = {}
    for nm, shp, dt in [
        ("sw1", [dh, dh], bf16), ("sw2", [dh, dh], bf16), ("sw3", [dh, dh], bf16),
        ("fw1", [2 * dh, dh], bf16), ("fw2", [dh, dh], bf16), ("fw3", [dh, dh], bf16),
        ("cb3", [2, 256], fp16), ("mlp_bias", [P, 4], f32),
        ("gs_wTh", [dh, 3 * dh], bf16), ("gs_wTx", [dx, 3 * dh], bf16),
        ("gs_bias", [P, 4], f32),
        ("gf_wTh", [dh, 3 * dh], bf16), ("gf_wTx", [dx, 3 * dh], bf16),
        ("gf_bias", [P, 4], f32),
    ]:
        w_d[nm] = nc.dram_tensor(nm, shp, dt, kind="ExternalInput")
    out_d = nc.dram_tensor("out_hfT", [P, meta["j_cols"]], f32, kind="ExternalOutput")

    Emax = max(lv["E"] for lv in levels)
    Jmax = max(lv["JP"] for lv in levels)
    a2a_out = [
        nc.dram_tensor(f"a2a_out{i}", [NCORES, P, 2, Emax], bf16)
        for i in range(2)
    ]

    with tile.TileContext(nc) as tc:
        with (
            tc.tile_pool(name="consts", bufs=1) as cst,
            tc.tile_pool(name="sb", bufs=2) as sb,
            tc.tile_pool(name="psA", bufs=2, space="PSUM") as psA,
            tc.tile_pool(name="psB", bufs=2, space="PSUM") as psB,
            tc.tile_pool(name="psC", bufs=2, space="PSUM") as psC,
            tc.tile_pool(name="dram", bufs=2, space="DRAM") as drp,
        ):
            # --- consts ---
            wt = {}
            for nm in ("sw1", "sw2", "sw3", "fw2", "fw3"):
                wt[nm] = cst.tile([dh, dh], bf16, tag=nm, name=nm)
                nc.sync.dma_start(out=wt[nm][:], in_=w_d[nm][:, :])
            wt["fw1a"] = cst.tile([dh, dh], bf16, tag="fw1a", name="fw1a")
            wt["fw1b"] = cst.tile([dh, dh], bf16, tag="fw1b", name="fw1b")
            nc.sync.dma_start(out=wt["fw1a"][:], in_=w_d["fw1"][0:dh, :])
            nc.sync.dma_start(out=wt["fw1b"][:], in_=w_d["fw1"][dh : 2 * dh, :])
            for nm in ("gs_wTh", "gf_wTh"):
                wt[nm] = cst.tile([dh, 3 * dh], bf16, tag=nm, name=nm)
                nc.sync.dma_start(out=wt[nm][:], in_=w_d[nm][:, :])
            for nm in ("gs_wTx", "gf_wTx"):
                wt[nm] = cst.tile([dx, 3 * dh], bf16, tag=nm, name=nm)
                nc.sync.dma_start(out=wt[nm][:], in_=w_d[nm][:, :])
            for nm in ("mlp_bias", "gs_bias", "gf_bias"):
                wt[nm] = cst.tile([P, 4], f32, tag=nm, name=nm)
                nc.sync.dma_start(out=wt[nm][:], in_=w_d[nm][:, :])
            wt["cb3"] = cst.tile([2, 256], fp16, tag="cb3", name="cb3")
            nc.sync.dma_start(out=wt["cb3"][:], in_=w_d["cb3"][:, :])

            from concourse.masks import make_identity
            ident = cst.tile([P, P], f32, tag="ident", name="ident")
            make_identity(nc, ident[:])
            iota_i = cst.tile([P, WIN], mybir.dt.int32, tag="iota_i", name="iota_i")
            nc.gpsimd.iota(iota_i[:], pattern=[[1, WIN]], base=0, channel_multiplier=0)
            iota_f = cst.tile([P, WIN], f32, tag="iota_f", name="iota_f")
            nc.vector.tensor_copy(iota_f[:], iota_i[:])

            mb = wt["mlp_bias"]
            nlv = len(levels)

            for li, lv in enumerate(levels):
                l, J, JP = lv["l"], lv["J"], lv["JP"]
                E, T, nw = lv["E"], lv["T"], lv["nw"]
                widths = lv["widths"]
                jo = lv["j_off"]
                last_level = li == nlv - 1
                ntw = lv["ntw"]

                # level metadata loads
                dst_sb = sb.tile([P, ntw], f32, tag="dst_sb", name="dst_sb")
                nc.scalar.dma_start(
                    out=dst_sb[:], in_=dst_d[:, lv["dst_off"] : lv["dst_off"] + ntw]
                )
                cnt_sb = sb.tile([2, JP], fp16, tag="cnt_sb", name="cnt_sb")
                nc.scalar.dma_start(out=cnt_sb[:], in_=cnt_d[:, jo : jo + JP])
                xT_sb = sb.tile([64, JP], bf16, tag="xT_sb", name="xT_sb")
                nc.scalar.dma_start(out=xT_sb[:], in_=xT_d[:, jo : jo + JP])
                if not last_level:
                    nxt = levels[li + 1]
                    nEn = NCORES * nxt["E"]
                    sidx_sb = sb.tile([P, nEn // 16], i16, tag="sidx", name="sidx")
                    nc.scalar.dma_start(
                        out=sidx_sb[:],
                        in_=idx_d[:, nxt["idx_off"] : nxt["idx_off"] + nEn // 16],
                    )

                # receive buffer: [128, 2, 8, E]
                rb = sb.tile([P, 2, NCORES, E], bf16, tag="rb", name="rb")
                if li == 0:
                    nc.sync.dma_start(
                        out=rb[:],
                        in_=recv0_d[:, :].rearrange(
                            "p (h k e) -> p h k e", h=2, k=NCORES
                        ),
                    )
                else:
                    nc.sync.dma_start(
                        out=rb[:],
                        in_=a2a_out[(l - 1) % 2][:, :, :, 0:E].rearrange(
                            "k p h e -> p h k e"
                        ),
                    )

                rm_sb = sb.tile([P, Jmax // P, 256], bf16, tag="rm_sb", name="rm_sb")
                hfout = sb.tile([P, Jmax], f32, tag="hfout", name="hfout")

                # flat edge-major views of the receive buffer
                hsT_all = rb[:, 0, :, :].rearrange("p k e -> p (k e)")
                hfT_all = rb[:, 1, :, :].rearrange("p k e -> p (k e)")

                union = lv["union"]
                wps = [None] * nw
                last_of_w = {}
                for ci, (t, w) in enumerate(union):
                    last_of_w[w] = (t, w)

                def start_window(w):
                    pS = psA.tile([P, WIN], f32, tag="msgaccS", name="msgaccS")
                    pF = psA.tile([P, WIN], f32, tag="msgaccF", name="msgaccF")
                    wd = widths[w]
                    only = all(pr[1] != w for pr in union)
                    nc.tensor.matmul(
                        out=pS[:, :wd], lhsT=wt["cb3"][:, 0:dh],
                        rhs=cnt_sb[:, w * WIN : w * WIN + wd],
                        start=True, stop=only,
                    )
                    nc.tensor.matmul(
                        out=pF[:, :wd], lhsT=wt["cb3"][:, dh : 2 * dh],
                        rhs=cnt_sb[:, w * WIN : w * WIN + wd],
                        start=True, stop=only,
                    )
                    wps[w] = (pS, pF)

                def gru(w):
                    wd = widths[w]
                    woff = w * WIN
                    pS, pF = wps[w]
                    for st, pm in (("s", pS), ("f", pF)):
                        wTh = wt["gs_wTh" if st == "s" else "gf_wTh"]
                        wTx = wt["gs_wTx" if st == "s" else "gf_wTx"]
                        gb = wt["gs_bias" if st == "s" else "gf_bias"]
                        msgT = sb.tile([P, WIN], bf16, tag="msgT", name="msgT")
                        nc.scalar.activation(
                            msgT[:, :wd], pm[:, :wd], mybir.ActivationFunctionType.Copy
                        )
                        pg = []
                        for gi in range(3):
                            pgi = psC.tile([P, WIN], f32, tag="gates", name="gates")
                            gsl = slice(gi * dh, (gi + 1) * dh)
                            nc.tensor.matmul(
                                out=pgi[:, :wd], lhsT=wTh[:, gsl], rhs=msgT[:, :wd],
                                start=True, stop=False,
                            )
                            nc.tensor.matmul(
                                out=pgi[:, :wd], lhsT=wTx[:dx, gsl],
                                rhs=xT_sb[:dx, woff : woff + wd],
                                start=False, stop=True,
                            )
                            pg.append(pgi)
                        r_sb = sb.tile([P, WIN], f32, tag="r_sb", name="r_sb")
                        nc.scalar.activation(
                            r_sb[:, :wd], pg[0][:, :wd],
                            mybir.ActivationFunctionType.Sigmoid, bias=gb[:, 0:1],
                        )
                        z_sb = sb.tile([P, WIN], f32, tag="z_sb", name="z_sb")
                        nc.scalar.activation(
                            z_sb[:, :wd], pg[1][:, :wd],
                            mybir.ActivationFunctionType.Sigmoid, bias=gb[:, 1:2],
                            scale=-1.0,
                        )
                        rb2 = sb.tile([P, WIN], f32, tag="rb2", name="rb2")
                        nc.vector.tensor_scalar_mul(rb2[:, :wd], r_sb[:, :wd], gb[:, 3:4])
                        npre = sb.tile([P, WIN], f32, tag="npre", name="npre")
                        nc.vector.tensor_tensor(
                            out=npre[:, :wd], in0=rb2[:, :wd], in1=pg[2][:, :wd],
                            op=mybir.AluOpType.add,
                        )
                        n_sb = sb.tile([P, WIN], f32, tag="n_sb", name="n_sb")
                        nc.scalar.activation(
                            n_sb[:, :wd], npre[:, :wd],
                            mybir.ActivationFunctionType.Tanh, bias=gb[:, 2:3],
                        )
                        if st == "f":
                            hN_ap = hfout[:, woff : woff + wd]
                        else:
                            hNs = sb.tile([P, WIN], f32, tag="hNs", name="hNs")
                            hN_ap = hNs[:, :wd]
                        nc.vector.tensor_tensor(
                            out=hN_ap, in0=n_sb[:, :wd], in1=z_sb[:, :wd],
                            op=mybir.AluOpType.mult,
                        )
                        nb = wd // P
                        csl = slice(0, dh) if st == "s" else slice(dh, 2 * dh)
                        tp = psB.tile([P, WIN], f32, tag="mlp", name="tp")
                        for b in range(nb):
                            nc.tensor.transpose(
                                out=tp[:, b * P : (b + 1) * P],
                                in_=hN_ap[:, b * P : (b + 1) * P]
                                if st == "s"
                                else hfout[:, woff + b * P : woff + (b + 1) * P],
                                identity=ident[:],
                            )
                        dst_rm = rm_sb[:, w * (WIN // P) : w * (WIN // P) + nb, csl]
                        if (w + (0 if st == "s" else 1)) % 2 == 0:
                            nc.scalar.activation(
                                dst_rm, tp[:, : nb * P],
                                mybir.ActivationFunctionType.Copy,
                            )
                        else:
                            nc.vector.tensor_copy(dst_rm, tp[:, : nb * P])

                # --- edge MLP groups ---
                ngroups = _ceil(T, 4)
                emitted = set()
                for g in range(ngroups):
                    t_lo = g * 4
                    t_hi = min(T, t_lo + 4)
                    gw = (t_hi - t_lo) * P
                    gsl = slice(t_lo * P, t_lo * P + gw)
                    hsT = hsT_all[:, gsl]
                    hfT = hfT_all[:, gsl]
                    # structural MLP
                    p1 = psB.tile([P, MGROUP], f32, tag="mlp", name="mlp")
                    nc.tensor.matmul(out=p1[:, :gw], lhsT=wt["sw1"][:], rhs=hsT)
                    h1 = sb.tile([P, MGROUP], bf16, tag="h1", name="h1")
                    nc.scalar.activation(
                        h1[:, :gw], p1[:, :gw], mybir.ActivationFunctionType.Relu,
                        bias=mb[:, 0:1],
                    )
                    p2 = psB.tile([P, MGROUP], f32, tag="mlp", name="mlp")
                    nc.tensor.matmul(out=p2[:, :gw], lhsT=wt["sw2"][:], rhs=h1[:, :gw])
                    h2 = sb.tile([P, MGROUP], bf16, tag="h2", name="h2")
                    nc.scalar.activation(
                        h2[:, :gw], p2[:, :gw], mybir.ActivationFunctionType.Relu,
                        bias=mb[:, 1:2],
                    )
                    p3 = psB.tile([P, MGROUP], f32, tag="mlp", name="mlp")
                    for t4 in range(t_hi - t_lo):
                        sl = slice(t4 * P, (t4 + 1) * P)
                        nc.tensor.matmul(
                            out=p3[:, sl], lhsT=h2[:, sl], rhs=wt["sw3"][:]
                        )
                    msgS = sb.tile([P, MGROUP], bf16, tag="msgS", name="msgS")
                    nc.vector.tensor_copy(msgS[:, :gw], p3[:, :gw])
                    # functional MLP
                    q1 = psB.tile([P, MGROUP], f32, tag="mlp", name="mlp")
                    nc.tensor.matmul(
                        out=q1[:, :gw], lhsT=wt["fw1a"][:], rhs=hsT,
                        start=True, stop=False,
                    )
                    nc.tensor.matmul(
                        out=q1[:, :gw], lhsT=wt["fw1b"][:], rhs=hfT,
                        start=False, stop=True,
                    )
                    f1 = sb.tile([P, MGROUP], bf16, tag="f1", name="f1")
                    nc.scalar.activation(
                        f1[:, :gw], q1[:, :gw], mybir.ActivationFunctionType.Relu,
                        bias=mb[:, 2:3],
                    )
                    q2 = psB.tile([P, MGROUP], f32, tag="mlp", name="mlp")
                    nc.tensor.matmul(out=q2[:, :gw], lhsT=wt["fw2"][:], rhs=f1[:, :gw])
                    f2 = sb.tile([P, MGROUP], bf16, tag="f2", name="f2")
                    nc.scalar.activation(
                        f2[:, :gw], q2[:, :gw], mybir.ActivationFunctionType.Relu,
                        bias=mb[:, 3:4],
                    )
                    q3 = psB.tile([P, MGROUP], f32, tag="mlp", name="mlp")
                    for t4 in range(t_hi - t_lo):
                        sl = slice(t4 * P, (t4 + 1) * P)
                        nc.tensor.matmul(
                            out=q3[:, sl], lhsT=f2[:, sl], rhs=wt["fw3"][:]
                        )
                    msgF = sb.tile([P, MGROUP], bf16, tag="msgF", name="msgF")
                    nc.vector.tensor_copy(msgF[:, :gw], q3[:, :gw])
                    # scatter per (tile, window)
                    for t4 in range(t_hi - t_lo):
                        t = t_lo + t4
                        for (tt, w) in union:
                            if tt != t:
                                continue
                            ci = union.index((t, w))
                            if wps[w] is None:
                                start_window(w)
                            wd = widths[w]
                            S = sb.tile([P, WIN], bf16, tag="onehot", name="onehot")
                            nc.vector.tensor_scalar(
                                S[:, :wd], iota_f[:, :wd], dst_sb[:, ci : ci + 1],
                                None, op0=mybir.AluOpType.is_equal,
                            )
                            last = last_of_w[w] == (t, w)
                            sl = slice(t4 * P, (t4 + 1) * P)
                            pS, pF = wps[w]
                            nc.tensor.matmul(
                                out=pS[:, :wd], lhsT=msgS[:, sl], rhs=S[:, :wd],
                                start=False, stop=last,
                            )
                            nc.tensor.matmul(
                                out=pF[:, :wd], lhsT=msgF[:, sl], rhs=S[:, :wd],
                                start=False, stop=last,
                            )
                            if last:
                                gru(w)
                # windows never touched by any tile
                for w in range(nw):
                    if wps[w] is None:
                        start_window(w)
                        gru(w)

                nc.sync.dma_start(out=out_d[:, jo : jo + JP], in_=hfout[:, :JP])

                # --- exchange for next level ---
                if not last_level:
                    nxt = levels[li + 1]
                    En = nxt["E"]
                    nEn = NCORES * En
                    a2a_in = drp.tile(
                        [NCORES, P, 2, En], bf16, tag="a2a_in", name="a2a_in"
                    )
                    assert nEn % 512 == 0 and 512 % En == 0
                    kper = 512 // En  # readers per 512-slot gather chunk
                    for c0 in range(0, nEn, 512):
                        k0 = c0 // En
                        sendc = sb.tile([P, 2, 512], bf16, tag="sendc", name="sendc")
                        nc.gpsimd.dma_gather(
                            out_ap=sendc[:],
                            in_ap=rm_sb[:, 0 : JP // P, :],
                            idxs_ap=sidx_sb[:, c0 // 16 : (c0 + 512) // 16],
                            num_idxs=512,
                            num_idxs_reg=512,
                            elem_size=256,
                            transpose=True,
                            sbuf_tokens_per_rank=P,
                            sbuf_free_dim_per_rank=512,
                        )
                        nc.sync.dma_start(
                            out=a2a_in[k0 : k0 + kper].rearrange(
                                "k p h e -> p h k e"
                            ),
                            in_=sendc[:].rearrange("p h (k e) -> p h k e", e=En),
                        )
                    nc.gpsimd.collective_compute(
                        "AllToAll",
                        mybir.AluOpType.bypass,
                        replica_groups=[list(range(NCORES))],
                        ins=[a2a_in[:]],
                        outs=[a2a_out[l % 2][:, :, :, 0:En]],
                    )
    nc.compile()
    return nc


def _assemble(meta, results):
    n, dh = meta["n"], meta["dh"]
    hf = np.zeros((n, dh), np.float32)
    for lv in meta["levels"]:
        for k in range(NCORES):
            real = lv["n"] // NCORES + (1 if k < lv["n"] % NCORES else 0)
            if real == 0:
                continue
            rows = lv["s"] + np.arange(real) * NCORES + k
            cols = results[k]["out_hfT"][:, lv["j_off"] : lv["j_off"] + real]
            hf[rows] = cols.T
    return hf


def build_and_run(inputs, trace=False, **kwargs):
    meta, percore, weights = _prep(inputs)
    nc = _build(meta)
    in_maps = [dict(percore[c], **weights) for c in range(NCORES)]
    res = run_bass_kernel_spmd(
        nc, in_maps, core_ids=list(range(NCORES)), trace=trace, **kwargs
    )
    return _assemble(meta, res.results), res


def kernel(**inputs):
    out, _ = build_and_run(inputs)
    return out


# revision 13
# speedup vs baseline: 6.3073x; 1.3837x over previous
"""DeepCell GNN message-passing kernel for 8 Trainium2 NeuronCores.

Levelized DAG recurrence (all cross edges source the immediately previous
level). v4 design — per-level targeted AllToAll instead of AllGather:

  - Nodes of each level are strided across the 8 cores (position p -> core
    p%8, local index p//8), flattening the per-(sender,reader) edge counts so
    every level's A2A stripe is E=128 edge slots.
  - Each core keeps its own level-l states [hs|hf] bf16 node-major in SBUF
    (rm_sb).  At level end it dma_gathers (SBUF-source) the per-edge source
    states every reader needs, in the reader's slot order, dumps them to an
    internal DRAM buffer and runs a collective AllToAll (8 stripes x 128
    slots x 512B = 0.5MB) instead of AllGather-ing all states (3.2MB).
  - Receivers consume their A2A stripes directly: the received buffer IS the
    per-edge [hs^T|hf^T] input of the message MLPs — no receiver gather.
  - Structural/functional MLPs, one-hot segment-sum scatter into per-window
    PSUM accumulators, GRU with h_prev=0, and the constant fold of
    same-level edges ([mlp(0); b3] x [cnt_same; cnt_cross]) are unchanged
    from the baseline kernel.
  - Level-1's "received" stripes are host-computed from the level-0 init
    states and fed as an input; levels 1..22 emit an A2A for the next level.
  - hf output is written per core as transposed f32 columns; host reassembles.
"""

import numpy as np
import ml_dtypes

import concourse.bass as bass
import concourse.bacc as bacc
import concourse.mybir as mybir
import concourse.tile as tile
from concourse.bass_utils import run_bass_kernel_spmd

NCORES = 8
P = 128
WIN = 512          # scatter/GRU window width (nodes)
MGROUP = 512       # edges per MLP group
f32 = mybir.dt.float32
bf16 = mybir.dt.bfloat16
fp16 = mybir.dt.float16
i16 = mybir.dt.int16

BF = ml_dtypes.bfloat16


def _ceil(a, b):
    return -(-a // b)


def _rup(a, b):
    return _ceil(a, b) * b


def _mlp_np(h, w1, b1, w2, b2, w3, b3):
    h = np.maximum(h @ w1 + b1, 0.0)
    h = np.maximum(h @ w2 + b2, 0.0)
    return h @ w3 + b3


def _pack_idx16(idx, ncols):
    """Pack an int16 index vector (len multiple of 16) into the
    [16-row wrapped, replicated to 128 partitions] dma_gather format."""
    n = idx.size
    assert n % 16 == 0 and n // 16 == ncols
    blk = idx.reshape(ncols, 16).T  # [16, ncols]; idx i -> row i%16, col i//16
    return np.tile(blk, (8, 1))     # [128, ncols]


def _prep(inputs):
    x = np.asarray(inputs["x"], np.float32)
    ei = np.asarray(inputs["edge_index"], np.int64)
    fl = np.asarray(inputs["forward_level"], np.int64)
    n = x.shape[0]
    dh = 128
    num_levels = int(fl.max()) + 1

    start = np.searchsorted(fl, np.arange(num_levels + 1)).astype(np.int64)
    src_all, tgt_all = ei[0], ei[1]
    tlv = fl[tgt_all]
    slv = fl[src_all]
    keep = tlv >= 1
    same = keep & (slv == tlv)
    cross = keep & (slv < tlv)
    assert (slv[cross] == tlv[cross] - 1).all(), "cross edges must source prev level"
    cnt_same = np.bincount(tgt_all[same], minlength=n).astype(np.int64)
    cnt_cross = np.bincount(tgt_all[cross], minlength=n).astype(np.int64)

    cs, ct = src_all[cross], tgt_all[cross]
    ct_lv = fl[ct]

    # ---- per-level structure (strided node->core assignment) ----
    levels = []
    idx_cols = 0
    dst_cols = 0
    j_cols = 0
    for l in range(1, num_levels):
        s_l, e_l = int(start[l]), int(start[l + 1])
        n_l = e_l - s_l
        assert n_l > 0
        J = _ceil(n_l, NCORES)
        JP = _rup(J, P)
        widths = []
        rem = JP
        while rem > 0:
            w = min(WIN, rem)
            widths.append(w)
            rem -= w
        m = ct_lv == l
        es, et = cs[m], ct[m]
        p_t = et - s_l                  # position within level l
        p_s = es - int(start[l - 1])    # position within level l-1
        rd = (p_t % NCORES).astype(np.int64)   # reader core
        sd = (p_s % NCORES).astype(np.int64)   # sender core
        d_loc = (p_t // NCORES).astype(np.int64)
        s_loc = (p_s // NCORES).astype(np.int64)
        pair = np.bincount(sd * NCORES + rd, minlength=NCORES * NCORES)
        # E must be a multiple of 32 so 4-reader gather chunks are 128-aligned
        E = _rup(max(int(pair.max()), 1), 32)
        assert E <= 128
        # per (sender k, reader j): slot lists
        send_idx = [np.zeros(NCORES * E, np.int16) for _ in range(NCORES)]
        dst_slot = [np.full(NCORES * E, -1.0, np.float32) for _ in range(NCORES)]
        order = np.lexsort((d_loc, rd, sd))
        es_o = order  # edges sorted by (sender, reader, dst)
        sd_o, rd_o = sd[es_o], rd[es_o]
        sl_o, dl_o = s_loc[es_o], d_loc[es_o]
        # slot within (k, j) group = running count
        key = sd_o * NCORES + rd_o
        # running position within each key group
        grp_start = np.searchsorted(key, np.arange(NCORES * NCORES))
        pos_in_grp = np.arange(key.size) - grp_start[key]
        slot = rd_o * E + pos_in_grp  # slot in SENDER k's send layout
        for k in range(NCORES):
            mk = sd_o == k
            send_idx[k][slot[mk]] = sl_o[mk].astype(np.int16)
        rslot = sd_o * E + pos_in_grp  # slot in READER j's receive layout
        for j in range(NCORES):
            mj = rd_o == j
            dst_slot[j][rslot[mj]] = dl_o[mj].astype(np.float32)
        T = NCORES * E // P
        # (tile, window) scatter descriptors: union across reader cores so the
        # emitted program (and dst_sb column order) is identical on all cores
        union = []
        for t in range(T):
            for w, wd in enumerate(widths):
                lo, hi = w * WIN, w * WIN + wd
                hit = False
                for j in range(NCORES):
                    dvals = dst_slot[j][t * P : (t + 1) * P]
                    if ((dvals >= lo) & (dvals < hi)).any():
                        hit = True
                        break
                if hit:
                    union.append((t, w))
        ntw = max(len(union), 1)
        levels.append(
            dict(
                l=l, s=s_l, n=n_l, J=J, JP=JP, widths=widths, nw=len(widths),
                E=E, T=T, send_idx=send_idx, dst_slot=dst_slot, union=union,
                ntw=ntw,
                idx_off=idx_cols, dst_off=dst_cols, j_off=j_cols,
            )
        )
        if l >= 2:
            # sender gather for this level's stripes is emitted at level l-1
            idx_cols += NCORES * E // 16
        dst_cols += ntw
        j_cols += JP

    # ---- per-core packed arrays ----
    percore = []
    for k in range(NCORES):
        idx16 = np.zeros((P, max(idx_cols, 1)), np.int16)
        dstc = np.full((P, max(dst_cols, 1)), -1.0, np.float32)
        cnt2 = np.zeros((2, max(j_cols, 1)), np.float16)
        xT = np.zeros((64, max(j_cols, 1)), BF)
        for lv in levels:
            l, E, T = lv["l"], lv["E"], lv["T"]
            if l >= 2:
                idx16[:, lv["idx_off"] : lv["idx_off"] + NCORES * E // 16] = (
                    _pack_idx16(lv["send_idx"][k], NCORES * E // 16)
                )
            # dst columns in the shared union order
            cols = np.full((P, lv["ntw"]), -1.0, np.float32)
            for ci, (t, w) in enumerate(lv["union"]):
                dv = lv["dst_slot"][k][t * P : (t + 1) * P].copy()
                lo, hi = w * WIN, w * WIN + lv["widths"][w]
                out = (dv < lo) | (dv >= hi)
                dv = dv - lo
                dv[out] = -1.0
                cols[:, ci] = dv
            dstc[:, lv["dst_off"] : lv["dst_off"] + lv["ntw"]] = cols
            # cnt2 + xT (strided rows)
            real = lv["n"] // NCORES + (1 if k < lv["n"] % NCORES else 0)
            jo = lv["j_off"]
            if real > 0:
                rows = lv["s"] + np.arange(real) * NCORES + k
                cnt2[0, jo : jo + real] = cnt_same[rows].astype(np.float16)
                cnt2[1, jo : jo + real] = cnt_cross[rows].astype(np.float16)
                xT[:, jo : jo + real] = x[rows].T.astype(BF)
        percore.append(dict(idx16=idx16, dstcol=dstc, cnt2=cnt2, xT=xT))

    # ---- level-0 init & level-1 receive stripes ----
    import jax

    n0 = int(start[1])
    cpu = jax.devices("cpu")[0]
    with jax.default_device(cpu):
        v = jax.random.uniform(jax.random.key(1), (n0, dh), np.float32) - 0.5
        v = v / np.linalg.norm(np.asarray(v), axis=1, keepdims=True)
    hs0 = np.asarray(v, np.float32)

    lv1 = levels[0]
    E1 = lv1["E"]
    for j in range(NCORES):
        buf = np.zeros((P, 2, NCORES, E1), BF)
        # reader j's stripe from sender k: source states of its (k, j) edges
        m = (ct_lv == 1)
        es1, et1 = cs[m], ct[m]
        p_t = et1 - lv1["s"]
        p_s = es1  # level-0 positions ARE global indices
        rdj = (p_t % NCORES) == j
        sdj = (p_s % NCORES).astype(np.int64)[rdj]
        s_glob = es1[rdj]
        d_loc = (p_t // NCORES)[rdj]
        o = np.lexsort((d_loc, sdj))
        sdj, s_glob = sdj[o], s_glob[o]
        gs = np.searchsorted(sdj, np.arange(NCORES))
        pos = np.arange(sdj.size) - gs[sdj]
        sl = sdj * E1 + pos
        st = hs0[s_glob].astype(BF)  # [cnt, 128]
        buf[:, 0, sl // E1, sl % E1] = st.T
        percore[j]["recv0"] = buf.reshape(P, 2 * NCORES * E1)

    # ---- weights ----
    g = lambda name: np.asarray(inputs[name], np.float32)
    sw1, sw2, sw3 = g("sw1"), g("sw2"), g("sw3")
    sb1, sb2, sb3 = g("sb1"), g("sb2"), g("sb3")
    fw1, fw2, fw3 = g("fw1"), g("fw2"), g("fw3")
    fb1, fb2, fb3 = g("fb1"), g("fb2"), g("fb3")
    c_s = _mlp_np(np.zeros((1, dh), np.float32), sw1, sb1, sw2, sb2, sw3, sb3)[0]
    c_f = _mlp_np(np.zeros((1, 2 * dh), np.float32), fw1, fb1, fw2, fb2, fw3, fb3)[0]
    cb3 = np.zeros((2, 256), np.float16)
    cb3[0, :dh] = c_s.astype(np.float16)
    cb3[1, :dh] = sb3.astype(np.float16)
    cb3[0, dh:] = c_f.astype(np.float16)
    cb3[1, dh:] = fb3.astype(np.float16)

    def gru_pack(wih, whh, bih, bhh):
        wih = np.asarray(wih, np.float32)
        bih = np.asarray(bih, np.float32)
        bhh = np.asarray(bhh, np.float32)
        wT_h = wih[:, :dh].T.astype(BF)          # [128, 384]
        wT_x = wih[:, dh:].T.astype(BF)          # [64, 384]
        bias = np.zeros((P, 4), np.float32)
        bias[:, 0] = bih[0:dh] + bhh[0:dh]       # r
        bias[:, 1] = -(bih[dh : 2 * dh] + bhh[dh : 2 * dh])  # -z bias
        bias[:, 2] = bih[2 * dh : 3 * dh]        # i_n bias
        bias[:, 3] = bhh[2 * dh : 3 * dh]        # h_n bias (scaled by r)
        return wT_h, wT_x, bias

    gs_wTh, gs_wTx, gs_bias = gru_pack(
        inputs["gs_wih"], inputs["gs_whh"], inputs["gs_bih"], inputs["gs_bhh"]
    )
    gf_wTh, gf_wTx, gf_bias = gru_pack(
        inputs["gf_wih"], inputs["gf_whh"], inputs["gf_bih"], inputs["gf_bhh"]
    )

    mlp_bias = np.zeros((P, 4), np.float32)
    mlp_bias[:, 0], mlp_bias[:, 1] = sb1, sb2
    mlp_bias[:, 2], mlp_bias[:, 3] = fb1, fb2

    weights = dict(
        sw1=sw1.astype(BF), sw2=sw2.astype(BF), sw3=sw3.astype(BF),
        fw1=fw1.astype(BF), fw2=fw2.astype(BF), fw3=fw3.astype(BF),
        cb3=cb3, mlp_bias=mlp_bias,
        gs_wTh=gs_wTh, gs_wTx=gs_wTx, gs_bias=gs_bias,
        gf_wTh=gf_wTh, gf_wTx=gf_wTx, gf_bias=gf_bias,
    )

    meta = dict(
        n=n, dh=dh, dx=64, n0=n0, levels=levels,
        idx_cols=max(idx_cols, 1), dst_cols=max(dst_cols, 1),
        j_cols=max(j_cols, 1), E1=E1,
    )
    return meta, percore, weights


def _build(meta):
    dh, dx = meta["dh"], meta["dx"]
    levels = meta["levels"]
    nc = bacc.Bacc("TRN2", target_bir_lowering=False, debug=False, num_devices=NCORES)

    idx_d = nc.dram_tensor("idx16", [P, meta["idx_cols"]], i16, kind="ExternalInput")
    dst_d = nc.dram_tensor("dstcol", [P, meta["dst_cols"]], f32, kind="ExternalInput")
    cnt_d = nc.dram_tensor("cnt2", [2, meta["j_cols"]], fp16, kind="ExternalInput")
    xT_d = nc.dram_tensor("xT", [64, meta["j_cols"]], bf16, kind="ExternalInput")
    E1 = meta["E1"]
    recv0_d = nc.dram_tensor(
        "recv0", [P, 2 * NCORES * E1], bf16, kind="ExternalInput"
    )
    w_d = {}
    for nm, shp, dt in [
        ("sw1", [dh, dh], bf16), ("sw2", [dh, dh], bf16), ("sw3", [dh, dh], bf16),
        ("fw1", [2 * dh, dh], bf16), ("fw2", [dh, dh], bf16), ("fw3", [dh, dh], bf16),
        ("cb3", [2, 256], fp16), ("mlp_bias", [P, 4], f32),
        ("gs_wTh", [dh, 3 * dh], bf16), ("gs_wTx", [dx, 3 * dh], bf16),
        ("gs_bias", [P, 4], f32),
        ("gf_wTh", [dh, 3 * dh], bf16), ("gf_wTx", [dx, 3 * dh], bf16),
        ("gf_bias", [P, 4], f32),
    ]:
        w_d[nm] = nc.dram_tensor(nm, shp, dt, kind="ExternalInput")
    out_d = nc.dram_tensor("out_hfT", [P, meta["j_cols"]], f32, kind="ExternalOutput")

    Emax = max(lv["E"] for lv in levels)
    Jmax = max(lv["JP"] for lv in levels)
    a2a_out = [
        nc.dram_tensor(f"a2a_out{i}", [NCORES, P, 2, Emax], bf16)
        for i in range(2)
    ]

    with tile.TileContext(nc) as tc:
        with (
            tc.tile_pool(name="consts", bufs=1) as cst,
            tc.tile_pool(name="sb", bufs=2) as sb,
            tc.tile_pool(name="psA", bufs=2, space="PSUM") as psA,
            tc.tile_pool(name="psB", bufs=2, space="PSUM") as psB,
            tc.tile_pool(name="psC", bufs=2, space="PSUM") as psC,
            tc.tile_pool(name="dram", bufs=2, space="DRAM") as drp,
        ):
            # --- consts ---
            wt = {}
            for nm in ("sw1", "sw2", "sw3", "fw2", "fw3"):
                wt[nm] = cst.tile([dh, dh], bf16, tag=nm, name=nm)
                nc.sync.dma_start(out=wt[nm][:], in_=w_d[nm][:, :])
            wt["fw1a"] = cst.tile([dh, dh], bf16, tag="fw1a", name="fw1a")
            wt["fw1b"] = cst.tile([dh, dh], bf16, tag="fw1b", name="fw1b")
            nc.sync.dma_start(out=wt["fw1a"][:], in_=w_d["fw1"][0:dh, :])
            nc.sync.dma_start(out=wt["fw1b"][:], in_=w_d["fw1"][dh : 2 * dh, :])
            for nm in ("gs_wTh", "gf_wTh"):
                wt[nm] = cst.tile([dh, 3 * dh], bf16, tag=nm, name=nm)
                nc.sync.dma_start(out=wt[nm][:], in_=w_d[nm][:, :])
            for nm in ("gs_wTx", "gf_wTx"):
                wt[nm] = cst.tile([dx, 3 * dh], bf16, tag=nm, name=nm)
                nc.sync.dma_start(out=wt[nm][:], in_=w_d[nm][:, :])
            for nm in ("mlp_bias", "gs_bias", "gf_bias"):
                wt[nm] = cst.tile([P, 4], f32, tag=nm, name=nm)
                nc.sync.dma_start(out=wt[nm][:], in_=w_d[nm][:, :])
            wt["cb3"] = cst.tile([2, 256], fp16, tag="cb3", name="cb3")
            nc.sync.dma_start(out=wt["cb3"][:], in_=w_d["cb3"][:, :])

            from concourse.masks import make_identity
            ident = cst.tile([P, P], f32, tag="ident", name="ident")
            make_identity(nc, ident[:])
            iota_i = cst.tile([P, WIN], mybir.dt.int32, tag="iota_i", name="iota_i")
            nc.gpsimd.iota(iota_i[:], pattern=[[1, WIN]], base=0, channel_multiplier=0)
            iota_f = cst.tile([P, WIN], f32, tag="iota_f", name="iota_f")
            nc.vector.tensor_copy(iota_f[:], iota_i[:])

            mb = wt["mlp_bias"]
            nlv = len(levels)

            for li, lv in enumerate(levels):
                l, J, JP = lv["l"], lv["J"], lv["JP"]
                E, T, nw = lv["E"], lv["T"], lv["nw"]
                widths = lv["widths"]
                jo = lv["j_off"]
                last_level = li == nlv - 1
                ntw = lv["ntw"]

                # level metadata loads
                dst_sb = sb.tile([P, ntw], f32, tag="dst_sb", name="dst_sb")
                nc.scalar.dma_start(
                    out=dst_sb[:], in_=dst_d[:, lv["dst_off"] : lv["dst_off"] + ntw]
                )
                cnt_sb = sb.tile([2, JP], fp16, tag="cnt_sb", name="cnt_sb")
                nc.scalar.dma_start(out=cnt_sb[:], in_=cnt_d[:, jo : jo + JP])
                xT_sb = sb.tile([64, JP], bf16, tag="xT_sb", name="xT_sb")
                nc.scalar.dma_start(out=xT_sb[:], in_=xT_d[:, jo : jo + JP])
                if not last_level:
                    nxt = levels[li + 1]
                    nEn = NCORES * nxt["E"]
                    sidx_sb = sb.tile([P, nEn // 16], i16, tag="sidx", name="sidx")
                    nc.scalar.dma_start(
                        out=sidx_sb[:],
                        in_=idx_d[:, nxt["idx_off"] : nxt["idx_off"] + nEn // 16],
                    )

                # receive buffer: [128, 2, 8, E]
                rb = sb.tile([P, 2, NCORES, E], bf16, tag="rb", name="rb")
                if li == 0:
                    nc.sync.dma_start(
                        out=rb[:],
                        in_=recv0_d[:, :].rearrange(
                            "p (h k e) -> p h k e", h=2, k=NCORES
                        ),
                    )
                else:
                    nc.sync.dma_start(
                        out=rb[:],
                        in_=a2a_out[(l - 1) % 2][:, :, :, 0:E].rearrange(
                            "k p h e -> p h k e"
                        ),
                    )

                rm_sb = sb.tile([P, Jmax // P, 256], bf16, tag="rm_sb", name="rm_sb")
                hfout = sb.tile([P, Jmax], f32, tag="hfout", name="hfout")

                # flat edge-major views of the receive buffer
                hsT_all = rb[:, 0, :, :].rearrange("p k e -> p (k e)")
                hfT_all = rb[:, 1, :, :].rearrange("p k e -> p (k e)")

                union = lv["union"]
                wps = [None] * nw
                last_of_w = {}
                for ci, (t, w) in enumerate(union):
                    last_of_w[w] = (t, w)

                def start_window(w):
                    pS = psA.tile([P, WIN], f32, tag="msgaccS", name="msgaccS")
                    pF = psA.tile([P, WIN], f32, tag="msgaccF", name="msgaccF")
                    wd = widths[w]
                    only = all(pr[1] != w for pr in union)
                    nc.tensor.matmul(
                        out=pS[:, :wd], lhsT=wt["cb3"][:, 0:dh],
                        rhs=cnt_sb[:, w * WIN : w * WIN + wd],
                        start=True, stop=only,
                    )
                    nc.tensor.matmul(
                        out=pF[:, :wd], lhsT=wt["cb3"][:, dh : 2 * dh],
                        rhs=cnt_sb[:, w * WIN : w * WIN + wd],
                        start=True, stop=only,
                    )
                    wps[w] = (pS, pF)

                def gru(w):
                    wd = widths[w]
                    woff = w * WIN
                    pS, pF = wps[w]
                    for st, pm in (("s", pS), ("f", pF)):
                        wTh = wt["gs_wTh" if st == "s" else "gf_wTh"]
                        wTx = wt["gs_wTx" if st == "s" else "gf_wTx"]
                        gb = wt["gs_bias" if st == "s" else "gf_bias"]
                        msgT = sb.tile([P, WIN], bf16, tag="msgT", name="msgT")
                        nc.scalar.activation(
                            msgT[:, :wd], pm[:, :wd], mybir.ActivationFunctionType.Copy
                        )
                        pg = []
                        for gi in range(3):
                            pgi = psC.tile([P, WIN], f32, tag="gates", name="gates")
                            gsl = slice(gi * dh, (gi + 1) * dh)
                            nc.tensor.matmul(
                                out=pgi[:, :wd], lhsT=wTh[:, gsl], rhs=msgT[:, :wd],
                                start=True, stop=False,
                            )
                            nc.tensor.matmul(
                                out=pgi[:, :wd], lhsT=wTx[:dx, gsl],
                                rhs=xT_sb[:dx, woff : woff + wd],
                                start=False, stop=True,
                            )
                            pg.append(pgi)
                        r_sb = sb.tile([P, WIN], f32, tag="r_sb", name="r_sb")
                        nc.scalar.activation(
                            r_sb[:, :wd], pg[0][:, :wd],
                            mybir.ActivationFunctionType.Sigmoid, bias=gb[:, 0:1],
                        )
                        z_sb = sb.tile([P, WIN], f32, tag="z_sb", name="z_sb")
                        nc.scalar.activation(
                            z_sb[:, :wd], pg[1][:, :wd],
                            mybir.ActivationFunctionType.Sigmoid, bias=gb[:, 1:2],
                            scale=-1.0,
                        )
                        rb2 = sb.tile([P, WIN], f32, tag="rb2", name="rb2")
                        nc.vector.tensor_scalar_mul(rb2[:, :wd], r_sb[:, :wd], gb[:, 3:4])
                        npre = sb.tile([P, WIN], f32, tag="npre", name="npre")
                        nc.vector.tensor_tensor(
                            out=npre[:, :wd], in0=rb2[:, :wd], in1=pg[2][:, :wd],
                            op=mybir.AluOpType.add,
                        )
                        n_sb = sb.tile([P, WIN], f32, tag="n_sb", name="n_sb")
                        nc.scalar.activation(
                            n_sb[:, :wd], npre[:, :wd],
                            mybir.ActivationFunctionType.Tanh, bias=gb[:, 2:3],
                        )
                        if st == "f":
                            hN_ap = hfout[:, woff : woff + wd]
                        else:
                            hNs = sb.tile([P, WIN], f32, tag="hNs", name="hNs")
                            hN_ap = hNs[:, :wd]
                        nc.vector.tensor_tensor(
                            out=hN_ap, in0=n_sb[:, :wd], in1=z_sb[:, :wd],
                            op=mybir.AluOpType.mult,
                        )
                        nb = wd // P
                        csl = slice(0, dh) if st == "s" else slice(dh, 2 * dh)
                        tp = psB.tile([P, WIN], f32, tag="mlp", name="tp")
                        for b in range(nb):
                            nc.tensor.transpose(
                                out=tp[:, b * P : (b + 1) * P],
                                in_=hN_ap[:, b * P : (b + 1) * P]
                                if st == "s"
                                else hfout[:, woff + b * P : woff + (b + 1) * P],
                                identity=ident[:],
                            )
                        dst_rm = rm_sb[:, w * (WIN // P) : w * (WIN // P) + nb, csl]
                        if (w + (0 if st == "s" else 1)) % 2 == 0:
                            nc.scalar.activation(
                                dst_rm, tp[:, : nb * P],
                                mybir.ActivationFunctionType.Copy,
                            )
                        else:
                            nc.vector.tensor_copy(dst_rm, tp[:, : nb * P])

                # --- edge MLP groups ---
                ngroups = _ceil(T, 4)
                emitted = set()
                for g in range(ngroups):
                    t_lo = g * 4
                    t_hi = min(T, t_lo + 4)
                    gw = (t_hi - t_lo) * P
                    gsl = slice(t_lo * P, t_lo * P + gw)
                    hsT = hsT_all[:, gsl]
                    hfT = hfT_all[:, gsl]
                    # structural MLP
                    p1 = psB.tile([P, MGROUP], f32, tag="mlp", name="mlp")
                    nc.tensor.matmul(out=p1[:, :gw], lhsT=wt["sw1"][:], rhs=hsT)
                    h1 = sb.tile([P, MGROUP], bf16, tag="h1", name="h1")
                    nc.scalar.activation(
                        h1[:, :gw], p1[:, :gw], mybir.ActivationFunctionType.Relu,
                        bias=mb[:, 0:1],
                    )
                    p2 = psB.tile([P, MGROUP], f32, tag="mlp", name="mlp")
                    nc.tensor.matmul(out=p2[:, :gw], lhsT=wt["sw2"][:], rhs=h1[:, :gw])
                    h2 = sb.tile([P, MGROUP], bf16, tag="h2", name="h2")
                    nc.scalar.activation(
                        h2[:, :gw], p2[:, :gw], mybir.ActivationFunctionType.Relu,
                        bias=mb[:, 1:2],
                    )
                    p3 = psB.tile([P, MGROUP], f32, tag="mlp", name="mlp")
                    for t4 in range(t_hi - t_lo):
                        sl = slice(t4 * P, (t4 + 1) * P)
                        nc.tensor.matmul(
                            out=p3[:, sl], lhsT=h2[:, sl], rhs=wt["sw3"][:]
                        )
                    msgS = sb.tile([P, MGROUP], bf16, tag="msgS", name="msgS")
                    nc.vector.tensor_copy(msgS[:, :gw], p3[:, :gw])
                    # functional MLP
                    q1 = psB.tile([P, MGROUP], f32, tag="mlp", name="mlp")
                    nc.tensor.matmul(
                        out=q1[:, :gw], lhsT=wt["fw1a"][:], rhs=hsT,
                        start=True, stop=False,
                    )
                    nc.tensor.matmul(
                        out=q1[:, :gw], lhsT=wt["fw1b"][:], rhs=hfT,
                        start=False, stop=True,
                    )
                    f1 = sb.tile([P, MGROUP], bf16, tag="f1", name="f1")
                    nc.scalar.activation(
                        f1[:, :gw], q1[:, :gw], mybir.ActivationFunctionType.Relu,
                        bias=mb[:, 2:3],
                    )
                    q2 = psB.tile([P, MGROUP], f32, tag="mlp", name="mlp")
                    nc.tensor.matmul(out=q2[:, :gw], lhsT=wt["fw2"][:], rhs=f1[:, :gw])
                    f2 = sb.tile([P, MGROUP], bf16, tag="f2", name="f2")
                    nc.scalar.activation(
                        f2[:, :gw], q2[:, :gw], mybir.ActivationFunctionType.Relu,
                        bias=mb[:, 3:4],
                    )
                    q3 = psB.tile([P, MGROUP], f32, tag="mlp", name="mlp")
                    for t4 in range(t_hi - t_lo):
                        sl = slice(t4 * P, (t4 + 1) * P)
                        nc.tensor.matmul(
                            out=q3[:, sl], lhsT=f2[:, sl], rhs=wt["fw3"][:]
                        )
                    msgF = sb.tile([P, MGROUP], bf16, tag="msgF", name="msgF")
                    nc.vector.tensor_copy(msgF[:, :gw], q3[:, :gw])
                    # scatter per (tile, window)
                    for t4 in range(t_hi - t_lo):
                        t = t_lo + t4
                        for (tt, w) in union:
                            if tt != t:
                                continue
                            ci = union.index((t, w))
                            if wps[w] is None:
                                start_window(w)
                            wd = widths[w]
                            S = sb.tile([P, WIN], bf16, tag="onehot", name="onehot")
                            nc.vector.tensor_scalar(
                                S[:, :wd], iota_f[:, :wd], dst_sb[:, ci : ci + 1],
                                None, op0=mybir.AluOpType.is_equal,
                            )
                            last = last_of_w[w] == (t, w)
                            sl = slice(t4 * P, (t4 + 1) * P)
                            pS, pF = wps[w]
                            nc.tensor.matmul(
                                out=pS[:, :wd], lhsT=msgS[:, sl], rhs=S[:, :wd],
                                start=False, stop=last,
                            )
                            nc.tensor.matmul(
                                out=pF[:, :wd], lhsT=msgF[:, sl], rhs=S[:, :wd],
                                start=False, stop=last,
                            )
                            if last:
                                gru(w)
                # windows never touched by any tile
                for w in range(nw):
                    if wps[w] is None:
                        start_window(w)
                        gru(w)

                nc.sync.dma_start(out=out_d[:, jo : jo + JP], in_=hfout[:, :JP])

                # --- exchange for next level ---
                if not last_level:
                    nxt = levels[li + 1]
                    En = nxt["E"]
                    nEn = NCORES * En
                    a2a_in = drp.tile(
                        [NCORES, P, 2, En], bf16, tag="a2a_in", name="a2a_in"
                    )
                    GCH = 4 * En  # 4 reader stripes per gather chunk (<=512)
                    for c0 in range(0, nEn, GCH):
                        k0 = c0 // En
                        sendc = sb.tile([P, 2, GCH], bf16, tag="sendc", name="sendc")
                        nc.gpsimd.dma_gather(
                            out_ap=sendc[:],
                            in_ap=rm_sb[:, 0 : JP // P, :],
                            idxs_ap=sidx_sb[:, c0 // 16 : (c0 + GCH) // 16],
                            num_idxs=GCH,
                            num_idxs_reg=GCH,
                            elem_size=256,
                            transpose=True,
                            sbuf_tokens_per_rank=P,
                            sbuf_free_dim_per_rank=512,
                        )
                        nc.sync.dma_start(
                            out=a2a_in[k0 : k0 + 4].rearrange(
                                "k p h e -> p h k e"
                            ),
                            in_=sendc[:].rearrange("p h (k e) -> p h k e", e=En),
                        )
                    nc.gpsimd.collective_compute(
                        "AllToAll",
                        mybir.AluOpType.bypass,
                        replica_groups=[list(range(NCORES))],
                        ins=[a2a_in[:]],
                        outs=[a2a_out[l % 2][:, :, :, 0:En]],
                    )
    nc.compile()
    return nc


def _assemble(meta, results):
    n, dh = meta["n"], meta["dh"]
    hf = np.zeros((n, dh), np.float32)
    for lv in meta["levels"]:
        for k in range(NCORES):
            real = lv["n"] // NCORES + (1 if k < lv["n"] % NCORES else 0)
            if real == 0:
                continue
            rows = lv["s"] + np.arange(real) * NCORES + k
            cols = results[k]["out_hfT"][:, lv["j_off"] : lv["j_off"] + real]
            hf[rows] = cols.T
    return hf


def build_and_run(inputs, trace=False, **kwargs):
    meta, percore, weights = _prep(inputs)
    nc = _build(meta)
    in_maps = [dict(percore[c], **weights) for c in range(NCORES)]
    res = run_bass_kernel_spmd(
        nc, in_maps, core_ids=list(range(NCORES)), trace=trace, **kwargs
    )
    return _assemble(meta, res.results), res


def kernel(**inputs):
    out, _ = build_and_run(inputs)
    return out
